# revision 5
# baseline (speedup 1.0000x reference)
"""Multi-head attention (B=4, S=2048, E=1024, H=16, D=64) on 8 NeuronCores.

Sharding: core_id = 2*b + g  (b in 0..3 batches, g in 0..1 head-groups of 8
heads = 512 features). Each core computes its batch's attention output for
its 8 heads plus that head-group's partial output projection. Host sums the
two partials per batch and adds the output bias.

Device kernel (per core, all matmuls in float32r at full PE rate):
  - QKV projections from pre-transposed x (xT: E on partitions).
    q is pre-scaled by 1/sqrt(D) host-side (folded into Wq/bq).
  - Scores computed transposed: S_T[tk, tq] = kT.T @ qT per head, with the
    two heads of a feature-chunk row-tiled (K=64 at base partitions 0/64).
  - softmax: exp on ScalarE straight from PSUM (no max-subtraction; scores
    are O(1) here), row-sums via an appended ones-column on V, attention
    mask folded host-side into multiplicative per-key V row weights exp(mask).
  - AV: outT[d, tq] accumulated over tk blocks in PSUM; normalization by
    the broadcast reciprocal of the ones-column row.
  - Output projection from outT against the pre-transposed Wo slice.
"""

import numpy as np

import concourse.mybir as mybir
import concourse.tile as tile
from concourse import bacc
from concourse.bass_utils import run_bass_kernel_spmd

F32 = mybir.dt.float32
F32R = mybir.dt.float32r

EMBED = 1024
HEADS_TOTAL = 16
HEAD_DIM = 64
BATCH = 4
SEQ = 2048
N_CORES = 8


def build_bass(S=SEQ, E=EMBED, F=512):
    """Bass program for one core: batch slice of S tokens, F=512 features
    (8 heads). Returns the compiled Bacc module."""
    P = 128
    D = HEAD_DIM
    H = F // D                  # heads per core (8)
    NPAIR = H // 2              # head pairs (4) == feature chunks of 128
    KC = E // P                 # contraction chunks for projections
    TQ = min(512, S)            # query tile
    TT = S // TQ                # query tiles
    KB = S // P                 # key blocks
    TCH = S // P                # token chunks
    NV = F + H                  # augmented V columns (65 per head)
    NVT = NV // 2               # 260: V projection free-dim split

    nc = bacc.Bacc(trn_type="TRN2")
    xT_d = nc.dram_tensor("xT", [E, S], F32R, kind="ExternalInput").ap()
    wq_d = nc.dram_tensor("wq", [E, F], F32R, kind="ExternalInput").ap()
    wk_d = nc.dram_tensor("wk", [E, F], F32R, kind="ExternalInput").ap()
    wv_d = nc.dram_tensor("wv", [E, NV], F32R, kind="ExternalInput").ap()
    wo_d = nc.dram_tensor("wo", [F, E], F32R, kind="ExternalInput").ap()
    bq_d = nc.dram_tensor("bq", [F], F32, kind="ExternalInput").ap()
    bk_d = nc.dram_tensor("bk", [F], F32, kind="ExternalInput").ap()
    bv_d = nc.dram_tensor("bv", [NV], F32, kind="ExternalInput").ap()
    wexp_d = nc.dram_tensor("wexp", [S], F32, kind="ExternalInput").ap()
    out_d = nc.dram_tensor("out", [S, E], F32, kind="ExternalOutput").ap()

    Exp = mybir.ActivationFunctionType.Exp
    mult = mybir.AluOpType.mult
    add = mybir.AluOpType.add

    with tile.TileContext(nc) as tc:
        import contextlib
        with contextlib.ExitStack() as ctx:
            consts = ctx.enter_context(tc.tile_pool(name="consts", bufs=1))
            small = ctx.enter_context(tc.tile_pool(name="small", bufs=2))
            qk_pool = ctx.enter_context(tc.tile_pool(name="qk", bufs=2))
            v_pool = ctx.enter_context(tc.tile_pool(name="v", bufs=1))
            pt_pool = ctx.enter_context(tc.tile_pool(name="pt", bufs=3))
            outT_pool = ctx.enter_context(tc.tile_pool(name="outT", bufs=1))
            pp = ctx.enter_context(tc.tile_pool(name="pp", bufs=2, space="PSUM"))
            ps_s = ctx.enter_context(tc.tile_pool(name="pss", bufs=2, space="PSUM"))
            ps_acc = ctx.enter_context(tc.tile_pool(name="psacc", bufs=2, space="PSUM"))

            # ---- constants ----
            bq_sb = consts.tile([P, F // P], F32)
            nc.sync.dma_start(out=bq_sb, in_=bq_d.rearrange("(c p) -> p c", p=P))
            bk_sb = consts.tile([P, F // P], F32)
            nc.sync.dma_start(out=bk_sb, in_=bk_d.rearrange("(c p) -> p c", p=P))
            bv_sb = consts.tile([1, NV], F32)
            nc.sync.dma_start(out=bv_sb, in_=bv_d[None, :])
            bvb_sb = consts.tile([P, NV], F32)
            nc.gpsimd.partition_broadcast(bvb_sb, bv_sb)
            wexp_sb = consts.tile([P, TCH], F32)
            nc.sync.dma_start(out=wexp_sb, in_=wexp_d.rearrange("(c p) -> p c", p=P))

            # ---- xT resident ----
            xT_pool = tc.tile_pool(name="xT", bufs=1)
            xTp = xT_pool.__enter__()
            xT_sb = xTp.tile([P, KC, S], F32R)
            xT_r = xT_d.rearrange("(k p) t -> p k t", p=P)
            for k in range(KC):
                nc.sync.dma_start(out=xT_sb[:, k], in_=xT_r[:, k])

            # ---- V projection (all pairs) ----
            wv_pool = tc.tile_pool(name="wv", bufs=1)
            wvp = wv_pool.__enter__()
            wv_sb = wvp.tile([P, KC, NV], F32R)
            nc.sync.dma_start(out=wv_sb, in_=wv_d.rearrange("(k p) n -> p k n", p=P))
            v_sb = v_pool.tile([P, TCH, NV], F32R, tag="v")
            for tch in range(TCH):
                for nt in range(2):
                    ps = pp.tile([P, 512], F32, tag="pp")
                    for k in range(KC):
                        nc.tensor.matmul(
                            ps[:, :NVT],
                            xT_sb[:, k, tch * P:(tch + 1) * P],
                            wv_sb[:, k, nt * NVT:(nt + 1) * NVT],
                            start=(k == 0), stop=(k == KC - 1),
                        )
                    dst = v_sb[:, tch, nt * NVT:(nt + 1) * NVT]
                    nc.vector.tensor_tensor(
                        dst, ps[:, :NVT], bvb_sb[:, nt * NVT:(nt + 1) * NVT], add)
                    nc.vector.tensor_scalar_mul(dst, dst, wexp_sb[:, tch:tch + 1])
            wv_pool.__exit__(None, None, None)

            # ---- per head-pair: K/Q projections then attention ----
            w_pool_cm = tc.tile_pool(name="w", bufs=2)
            w_pool = w_pool_cm.__enter__()
            wq_r = wq_d.rearrange("(k p) f -> p k f", p=P)
            wk_r = wk_d.rearrange("(k p) f -> p k f", p=P)
            outT_tiles = []
            for pr in range(NPAIR):
                fsl = slice(pr * P, (pr + 1) * P)
                wk_t = w_pool.tile([P, KC, P], F32R, tag="wk")
                nc.sync.dma_start(out=wk_t, in_=wk_r[:, :, fsl])
                wq_t = w_pool.tile([P, KC, P], F32R, tag="wq")
                nc.sync.dma_start(out=wq_t, in_=wq_r[:, :, fsl])

                kT_t = qk_pool.tile([P, S], F32R, tag="kT")
                qT_t = qk_pool.tile([P, S], F32R, tag="qT")
                for dst, w_t, b_sb in ((kT_t, wk_t, bk_sb), (qT_t, wq_t, bq_sb)):
                    for tt in range(TT):
                        tsl = slice(tt * TQ, (tt + 1) * TQ)
                        ps = pp.tile([P, 512], F32, tag="pp")
                        for k in range(KC):
                            nc.tensor.matmul(
                                ps[:, :TQ], w_t[:, k], xT_sb[:, k, tsl],
                                start=(k == 0), stop=(k == KC - 1))
                        nc.vector.tensor_scalar_add(dst[:, tsl], ps[:, :TQ],
                                                    b_sb[:, pr:pr + 1])

                outT_t = outT_pool.tile([P, S], F32R, tag=f"outT{pr}")
                outT_tiles.append(outT_t)
                for tt in range(TT):
                    tsl = slice(tt * TQ, (tt + 1) * TQ)
                    acc_e = ps_acc.tile([P, TQ], F32, tag="acc")
                    acc_o = ps_acc.tile([P, TQ], F32, tag="acc")
                    for tkb in range(KB):
                        ksl = slice(tkb * P, (tkb + 1) * P)
                        sc = ps_s.tile([P, 2, TQ], F32, tag="s")
                        nc.tensor.matmul(sc[:, 0], kT_t[0:64, ksl], qT_t[0:64, tsl],
                                         start=True, stop=True)
                        nc.tensor.matmul(sc[:, 1], kT_t[64:128, ksl], qT_t[64:128, tsl],
                                         start=True, stop=True)
                        pt = pt_pool.tile([P, 2, TQ], F32R, tag="pt")
                        nc.scalar.activation(pt, sc, Exp)
                        c0 = pr * 2 * (D + 1)
                        nc.tensor.matmul(acc_e[0:D + 1], v_sb[:, tkb, c0:c0 + D + 1],
                                         pt[:, 0], start=(tkb == 0), stop=(tkb == KB - 1))
                        nc.tensor.matmul(acc_o[0:D + 1],
                                         v_sb[:, tkb, c0 + D + 1:c0 + 2 * (D + 1)],
                                         pt[:, 1], start=(tkb == 0), stop=(tkb == KB - 1))
                    for hh, acc in ((0, acc_e), (1, acc_o)):
                        rec = small.tile([1, TQ], F32, tag="rec")
                        nc.vector.reciprocal(rec, acc[D:D + 1, :])
                        bc = small.tile([P, TQ], F32, tag="bc")
                        nc.gpsimd.partition_broadcast(bc, rec)
                        nc.vector.tensor_tensor(
                            outT_t[hh * D:(hh + 1) * D, tsl],
                            acc[0:D, :], bc[0:D, :], mult)

            w_pool_cm.__exit__(None, None, None)
            xT_pool.__exit__(None, None, None)

            # ---- output projection ----
            wo_pool = ctx.enter_context(tc.tile_pool(name="wo", bufs=1))
            ostage_pool = ctx.enter_context(tc.tile_pool(name="ostage", bufs=3))
            wo_sb = wo_pool.tile([P, NPAIR, E], F32R)
            nc.sync.dma_start(out=wo_sb, in_=wo_d.rearrange("(c p) o -> p c o", p=P))
            for tch in range(TCH):
                tsl = slice(tch * P, (tch + 1) * P)
                ostage = ostage_pool.tile([P, E], F32, tag="ostage")
                OW = min(512, E)
                for nt in range(E // OW):
                    ps = pp.tile([P, 512], F32, tag="pp")
                    for c in range(NPAIR):
                        nc.tensor.matmul(
                            ps[:, :OW], outT_tiles[c][:, tsl],
                            wo_sb[:, c, nt * OW:(nt + 1) * OW],
                            start=(c == 0), stop=(c == NPAIR - 1))
                    nc.vector.tensor_copy(out=ostage[:, nt * OW:(nt + 1) * OW],
                                          in_=ps[:, :OW])
                nc.sync.dma_start(out=out_d[tsl, :], in_=ostage)

    nc.compile()
    return nc


_CACHE = {}


def _get_nc(S, E, F):
    key = (S, E, F)
    if key not in _CACHE:
        _CACHE[key] = build_bass(S, E, F)
    return _CACHE[key]


def make_in_maps(x, mask, Wq, bq, Wk, bk, Wv, bv, Wo):
    """Host-side shard/layout prep. Returns list of 8 per-core input dicts."""
    B, S, E = x.shape
    D = HEAD_DIM
    scale = np.float32(1.0 / np.sqrt(D))
    F = Wq.shape[0] // 2          # features per head-group
    H = F // D

    f32 = np.float32
    xT = [np.ascontiguousarray(x[b].T).astype(f32, copy=False) for b in range(B)]
    mask_w = [np.exp(mask[b, 0, 0, :]).astype(f32) for b in range(B)]

    per_g = []
    for g in range(2):
        rows = slice(g * F, (g + 1) * F)
        wq_g = np.ascontiguousarray((Wq[rows, :] * scale).T).astype(f32, copy=False)
        bq_g = (bq[rows] * scale).astype(f32)
        wk_g = np.ascontiguousarray(Wk[rows, :].T).astype(f32, copy=False)
        bk_g = bk[rows].astype(f32)
        WvT = Wv[rows, :].T                      # (E, F)
        wv_g = np.zeros((E, F + H), dtype=f32)
        bv_g = np.zeros(F + H, dtype=f32)
        for h in range(H):
            wv_g[:, h * (D + 1):h * (D + 1) + D] = WvT[:, h * D:(h + 1) * D]
            bv_g[h * (D + 1):h * (D + 1) + D] = bv[rows][h * D:(h + 1) * D]
            bv_g[h * (D + 1) + D] = 1.0
        wo_g = np.ascontiguousarray(Wo[:, rows].T).astype(f32, copy=False)
        per_g.append((wq_g, bq_g, wk_g, bk_g, wv_g, bv_g, wo_g))

    in_maps = []
    for b in range(B):
        for g in range(2):
            wq_g, bq_g, wk_g, bk_g, wv_g, bv_g, wo_g = per_g[g]
            in_maps.append({
                "xT": xT[b], "wq": wq_g, "wk": wk_g, "wv": wv_g, "wo": wo_g,
                "bq": bq_g, "bk": bk_g, "bv": bv_g, "wexp": mask_w[b],
            })
    return in_maps


def kernel(x, mask, Wq, bq, Wk, bk, Wv, bv, Wo, bo, _results_out=None):
    x = np.asarray(x, dtype=np.float32)
    mask = np.asarray(mask, dtype=np.float32)
    B, S, E = x.shape
    nc = _get_nc(S, E, Wq.shape[0] // 2)
    in_maps = make_in_maps(x, mask, np.asarray(Wq), np.asarray(bq),
                           np.asarray(Wk), np.asarray(bk),
                           np.asarray(Wv), np.asarray(bv), np.asarray(Wo))
    res = run_bass_kernel_spmd(nc, in_maps, core_ids=list(range(N_CORES)))
    if _results_out is not None:
        _results_out.append(res)
    out = np.empty((B, S, E), dtype=np.float32)
    bo32 = np.asarray(bo, dtype=np.float32)
    for b in range(B):
        out[b] = res.results[2 * b]["out"] + res.results[2 * b + 1]["out"] + bo32
    return out


# revision 33
# speedup vs baseline: 1.0351x; 1.0351x over previous
"""Multi-head attention (B=4, S=2048, E=1024, H=16, D=64) on 8 NeuronCores.

Sharding: core_id = 2*b + g  (b in 0..3 batches, g in 0..1 head-groups of 8
heads = 512 features). Each core computes its batch's attention output for
its 8 heads plus that head-group's partial output projection. Host sums the
two partials per batch and adds the output bias.

Device kernel (per core, all matmuls in float32r at full PE rate):
  - QKV projections from pre-transposed x (xT: E on partitions).
    q is pre-scaled by 1/sqrt(D) host-side (folded into Wq/bq).
  - Scores computed transposed: S_T[tk, tq] = kT.T @ qT per head, with the
    two heads of a feature-chunk row-tiled (K=64 at base partitions 0/64).
  - softmax: exp on ScalarE straight from PSUM (no max-subtraction; scores
    are O(1) here), row-sums via an appended ones-column on V, attention
    mask folded host-side into multiplicative per-key V row weights exp(mask).
  - AV: outT[d, tq] accumulated over tk blocks in PSUM; normalization by
    the broadcast reciprocal of the ones-column row.
  - Output projection from outT against the pre-transposed Wo slice,
    interleaved into the last head-pair's attention.
"""

import contextlib

import numpy as np

import concourse.mybir as mybir
import concourse.tile as tile
from concourse import bacc
from concourse.bass_utils import run_bass_kernel_spmd

F32 = mybir.dt.float32
F32R = mybir.dt.float32r

EMBED = 1024
HEADS_TOTAL = 16
HEAD_DIM = 64
BATCH = 4
SEQ = 2048
N_CORES = 8


def build_bass(S=SEQ, E=EMBED, F=512, reps=1, stages="all"):
    """Bass program for one core: batch slice of S tokens, F=512 features
    (8 heads). Returns the compiled Bacc module.

    reps>1 wraps the compute body (sans output projection) in a For_i loop —
    a timing harness used to measure per-iteration HW time via wall deltas.
    stages (reps mode only): all | proj | attn | attn_tinyexp | attn_noav."""
    P = 128
    D = HEAD_DIM
    H = F // D                  # heads per core (8)
    NPAIR = H // 2              # head pairs (4) == feature chunks of 128
    KC = E // P                 # contraction chunks for projections
    TQ = min(512, S)            # query tile
    TT = S // TQ                # query tiles
    KB = S // P                 # key blocks
    TCH = S // P                # token chunks
    NV = F + H                  # augmented V columns (65 per head)
    GV = NV // 2                # 260: V column group (2 pairs) per projection
    OW = min(512, E)            # output projection free tile

    nc = bacc.Bacc(trn_type="TRN2")
    xT_d = nc.dram_tensor("xT", [E, S], F32R, kind="ExternalInput").ap()
    wq_d = nc.dram_tensor("wq", [E, F], F32R, kind="ExternalInput").ap()
    wk_d = nc.dram_tensor("wk", [E, F], F32R, kind="ExternalInput").ap()
    wv_d = nc.dram_tensor("wv", [E, NV], F32R, kind="ExternalInput").ap()
    wo_d = nc.dram_tensor("wo", [F, E], F32R, kind="ExternalInput").ap()
    bq_d = nc.dram_tensor("bq", [F], F32, kind="ExternalInput").ap()
    bk_d = nc.dram_tensor("bk", [F], F32, kind="ExternalInput").ap()
    bv_d = nc.dram_tensor("bv", [NV], F32, kind="ExternalInput").ap()
    wexp_d = nc.dram_tensor("wexp", [S], F32, kind="ExternalInput").ap()
    out_d = nc.dram_tensor("out", [S, E], F32, kind="ExternalOutput").ap()

    Exp = mybir.ActivationFunctionType.Exp
    mult = mybir.AluOpType.mult
    add = mybir.AluOpType.add

    with tile.TileContext(nc) as tc:
        with contextlib.ExitStack() as ctx:
            consts = ctx.enter_context(tc.tile_pool(name="consts", bufs=1))
            small = ctx.enter_context(tc.tile_pool(name="small", bufs=2))
            qk_bufs = 8 if stages.startswith("attn") else 2
            qk_pool = ctx.enter_context(tc.tile_pool(name="qk", bufs=qk_bufs))
            v_pool = ctx.enter_context(tc.tile_pool(name="v", bufs=1))
            pt_pool = ctx.enter_context(tc.tile_pool(name="pt", bufs=3))
            outT_pool = ctx.enter_context(tc.tile_pool(name="outT", bufs=1))
            pp = ctx.enter_context(tc.tile_pool(name="pp", bufs=2, space="PSUM"))
            ps_s = ctx.enter_context(tc.tile_pool(name="pss", bufs=2, space="PSUM"))
            ps_acc = ctx.enter_context(tc.tile_pool(name="psacc", bufs=2, space="PSUM"))

            # ---- constants ----
            bq_sb = consts.tile([P, F // P], F32)
            nc.sync.dma_start(out=bq_sb, in_=bq_d.rearrange("(c p) -> p c", p=P))
            bk_sb = consts.tile([P, F // P], F32)
            nc.sync.dma_start(out=bk_sb, in_=bk_d.rearrange("(c p) -> p c", p=P))
            bv_sb = consts.tile([1, NV], F32)
            nc.sync.dma_start(out=bv_sb, in_=bv_d[None, :])
            bvb_sb = consts.tile([P, NV], F32)
            nc.gpsimd.partition_broadcast(bvb_sb, bv_sb)
            wexp_sb = consts.tile([P, TCH], F32)
            nc.sync.dma_start(out=wexp_sb, in_=wexp_d.rearrange("(c p) -> p c", p=P))

            # ---- long-lived activation tiles ----
            v_sb = v_pool.tile([P, TCH, NV], F32R, tag="v")
            outT_tiles = [outT_pool.tile([P, S], F32R, tag=f"outT{pr}",
                                         name=f"outT{pr}")
                          for pr in range(NPAIR)]

            # ---- xT + weight pools (freed before the last pair's attention) --
            if stages.startswith("attn"):
                KC = KC // 2        # timing variant: halve projection work
            xT_pool_cm = tc.tile_pool(name="xT", bufs=1)
            xTp = xT_pool_cm.__enter__()
            xT_sb = xTp.tile([P, KC, S], F32R)
            xT_r = xT_d.rearrange("(k p) t -> p k t", p=P)
            for k in range(KC):
                nc.sync.dma_start(out=xT_sb[:, k], in_=xT_r[:, k])

            w_pool_cm = tc.tile_pool(name="w", bufs=2)
            w_pool = w_pool_cm.__enter__()
            wq_r = wq_d.rearrange("(k p) f -> p k f", p=P)
            wk_r = wk_d.rearrange("(k p) f -> p k f", p=P)
            wv_r = wv_d.rearrange("(k p) n -> p k n", p=P)

            def project_v_group(grp):
                """V projection for column group grp (pairs 2*grp, 2*grp+1)."""
                csl = slice(grp * GV, (grp + 1) * GV)
                wv_t = w_pool.tile([P, KC, GV], F32R, tag="wv", bufs=1)
                nc.sync.dma_start(out=wv_t, in_=wv_r[:, :, csl])
                for tch in range(TCH):
                    ps = pp.tile([P, 512], F32, tag="pp")
                    for k in range(KC):
                        nc.tensor.matmul(
                            ps[:, :GV],
                            xT_sb[:, k, tch * P:(tch + 1) * P],
                            wv_t[:, k],
                            start=(k == 0), stop=(k == KC - 1),
                        )
                    dst = v_sb[:, tch, csl]
                    nc.vector.tensor_tensor(dst, ps[:, :GV], bvb_sb[:, csl], add)
                    nc.vector.tensor_scalar_mul(dst, dst, wexp_sb[:, tch:tch + 1])

            def project_kq(pr):
                fsl = slice(pr * P, (pr + 1) * P)
                wk_t = w_pool.tile([P, KC, P], F32R, tag="wk", bufs=1)
                nc.sync.dma_start(out=wk_t, in_=wk_r[:, :, fsl])
                wq_t = w_pool.tile([P, KC, P], F32R, tag="wq", bufs=1)
                nc.sync.dma_start(out=wq_t, in_=wq_r[:, :, fsl])
                kT_t = qk_pool.tile([P, S], F32R, tag="kT")
                qT_t = qk_pool.tile([P, S], F32R, tag="qT")
                for dst, w_t, b_sb in ((kT_t, wk_t, bk_sb), (qT_t, wq_t, bq_sb)):
                    for tt in range(TT):
                        tsl = slice(tt * TQ, (tt + 1) * TQ)
                        ps = pp.tile([P, 512], F32, tag="pp")
                        for k in range(KC):
                            nc.tensor.matmul(
                                ps[:, :TQ], w_t[:, k], xT_sb[:, k, tsl],
                                start=(k == 0), stop=(k == KC - 1))
                        nc.vector.tensor_scalar_add(dst[:, tsl], ps[:, :TQ],
                                                    b_sb[:, pr:pr + 1])
                return kT_t, qT_t

            # wo/ostage pools are entered late (reuse xT/w space) but vars
            # are bound here for the closures below.
            wo_sb = None
            ostage_pool = None

            def oproj_tile(tch):
                """Output projection for token chunk tch (needs all outT)."""
                tsl = slice(tch * P, (tch + 1) * P)
                ostage = ostage_pool.tile([P, E], F32, tag="ostage")
                for nt in range(E // OW):
                    ps = pp.tile([P, 512], F32, tag="pp")
                    for c in range(NPAIR):
                        nc.tensor.matmul(
                            ps[:, :OW], outT_tiles[c][:, tsl],
                            wo_sb[:, c, nt * OW:(nt + 1) * OW],
                            start=(c == 0), stop=(c == NPAIR - 1))
                    nc.vector.tensor_copy(out=ostage[:, nt * OW:(nt + 1) * OW],
                                          in_=ps[:, :OW])
                nc.sync.dma_start(out=out_d[tsl, :], in_=ostage)

            def attention_pair(pr, kT_t, qT_t, with_oproj):
                outT_t = outT_tiles[pr]
                c0 = pr * 2 * (D + 1)
                do_av = stages != "attn_noav"
                for tt in range(TT):
                    tsl = slice(tt * TQ, (tt + 1) * TQ)
                    acc_e = ps_acc.tile([P, TQ], F32, tag="acc")
                    acc_o = ps_acc.tile([P, TQ], F32, tag="acc")
                    for tkb in range(KB):
                        ksl = slice(tkb * P, (tkb + 1) * P)
                        sc = ps_s.tile([P, 2, TQ], F32, tag="s")
                        nc.tensor.matmul(sc[:, 0], kT_t[0:64, ksl],
                                         qT_t[0:64, tsl], start=True, stop=True)
                        nc.tensor.matmul(sc[:, 1], kT_t[64:128, ksl],
                                         qT_t[64:128, tsl], start=True, stop=True)
                        pt = pt_pool.tile([P, 2, TQ], F32R, tag="pt")
                        if stages == "attn_tinyexp":
                            nc.scalar.activation(pt[:, :, :8], sc[:, :, :8], Exp)
                        else:
                            nc.scalar.activation(pt, sc, Exp)
                        if do_av:
                            nc.tensor.matmul(acc_e[0:D + 1],
                                             v_sb[:, tkb, c0:c0 + D + 1],
                                             pt[:, 0], start=(tkb == 0),
                                             stop=(tkb == KB - 1))
                            nc.tensor.matmul(acc_o[0:D + 1],
                                             v_sb[:, tkb, c0 + D + 1:c0 + 2 * (D + 1)],
                                             pt[:, 1], start=(tkb == 0),
                                             stop=(tkb == KB - 1))
                    if not do_av:
                        continue
                    for hh, acc in ((0, acc_e), (1, acc_o)):
                        if stages == "attn_nonorm":
                            nc.vector.tensor_copy(
                                out=outT_t[hh * D:(hh + 1) * D, tsl],
                                in_=acc[0:D, :])
                            continue
                        # free the PSUM accumulator with two fast copies;
                        # the recip chain then runs entirely from SBUF.
                        stg = small.tile([D, TQ], F32, tag="stg")
                        nc.vector.tensor_copy(out=stg, in_=acc[0:D, :])
                        rc = small.tile([P, TQ], F32, tag="rc")
                        nc.vector.tensor_copy(out=rc[0:1, :], in_=acc[D:D + 1, :])
                        bc = small.tile([P, TQ], F32, tag="bc")
                        nc.gpsimd.partition_broadcast(bc[0:D], rc[0:1, :])
                        nc.vector.reciprocal_approx_fast(rc[0:D], bc[0:D])
                        nc.vector.tensor_tensor(
                            outT_t[hh * D:(hh + 1) * D, tsl],
                            stg, rc[0:D, :], mult)
                    if with_oproj:
                        for tch in range(tt * (TCH // TT), (tt + 1) * (TCH // TT)):
                            oproj_tile(tch)

            # ---- emission schedule ----
            if reps > 1:
                # timing harness: loop the compute body (sans output
                # projection); keep xT/w pools open across iterations.
                if stages.startswith("attn"):
                    project_v_group(0)
                    project_v_group(1)
                    kqs = [project_kq(pr) for pr in range(2)]
                    w_pool_cm.__exit__(None, None, None)
                    xT_pool_cm.__exit__(None, None, None)
                    with tc.For_i(0, reps, 1):
                        # same unit count as the real kernel (4 pair-passes)
                        for pr in (0, 1, 0, 1):
                            attention_pair(pr, *kqs[pr], with_oproj=False)
                elif stages == "proj":
                    with tc.For_i(0, reps, 1):
                        project_v_group(0)
                        project_v_group(1)
                        for pr in range(NPAIR):
                            project_kq(pr)
                else:
                    with tc.For_i(0, reps, 1):
                        project_v_group(0)
                        kq0 = project_kq(0)
                        project_v_group(1)
                        attention_pair(0, *kq0, with_oproj=False)
                        for pr in range(1, NPAIR):
                            kqp = project_kq(pr)
                            attention_pair(pr, *kqp, with_oproj=False)
                # dummy output write so the NEFF has a valid output
                if not stages.startswith("attn"):
                    w_pool_cm.__exit__(None, None, None)
                    xT_pool_cm.__exit__(None, None, None)
                ostage_pool = ctx.enter_context(
                    tc.tile_pool(name="ostage", bufs=1))
                for tch in range(TCH):
                    ostage = ostage_pool.tile([P, E], F32, tag="ostage")
                    nc.vector.tensor_copy(out=ostage[:, :P],
                                          in_=v_sb[:, tch, :P])
                    nc.sync.dma_start(out=out_d[tch * P:(tch + 1) * P, :E],
                                      in_=ostage)
            else:
                project_v_group(0)
                kq = {0: project_kq(0)}
                attention_pair(0, *kq[0], with_oproj=False)
                kq[1] = project_kq(1)
                attention_pair(1, *kq[1], with_oproj=False)
                project_v_group(1)
                kq[2] = project_kq(2)
                attention_pair(2, *kq[2], with_oproj=False)
                kq[3] = project_kq(3)

                # free xT + weight staging before the final pair; load wo into
                # the freed space and interleave the output projection.
                w_pool_cm.__exit__(None, None, None)
                xT_pool_cm.__exit__(None, None, None)

                wo_pool = ctx.enter_context(tc.tile_pool(name="wo", bufs=1))
                ostage_pool = ctx.enter_context(
                    tc.tile_pool(name="ostage", bufs=3))
                wo_sb = wo_pool.tile([P, NPAIR, E], F32R)
                nc.sync.dma_start(out=wo_sb,
                                  in_=wo_d.rearrange("(c p) o -> p c o", p=P))

                attention_pair(3, *kq[3], with_oproj=True)

    nc.compile()
    return nc


_CACHE = {}


def _get_nc(S, E, F):
    key = (S, E, F)
    if key not in _CACHE:
        _CACHE[key] = build_bass(S, E, F)
    return _CACHE[key]


def make_in_maps(x, mask, Wq, bq, Wk, bk, Wv, bv, Wo):
    """Host-side shard/layout prep. Returns list of 8 per-core input dicts."""
    B, S, E = x.shape
    D = HEAD_DIM
    scale = np.float32(1.0 / np.sqrt(D))
    F = Wq.shape[0] // 2          # features per head-group
    H = F // D

    f32 = np.float32
    xT = [np.ascontiguousarray(x[b].T).astype(f32, copy=False) for b in range(B)]
    mask_w = [np.exp(mask[b, 0, 0, :]).astype(f32) for b in range(B)]

    per_g = []
    for g in range(2):
        rows = slice(g * F, (g + 1) * F)
        wq_g = np.ascontiguousarray((Wq[rows, :] * scale).T).astype(f32, copy=False)
        bq_g = (bq[rows] * scale).astype(f32)
        wk_g = np.ascontiguousarray(Wk[rows, :].T).astype(f32, copy=False)
        bk_g = bk[rows].astype(f32)
        WvT = Wv[rows, :].T                      # (E, F)
        wv_g = np.zeros((E, F + H), dtype=f32)
        bv_g = np.zeros(F + H, dtype=f32)
        for h in range(H):
            wv_g[:, h * (D + 1):h * (D + 1) + D] = WvT[:, h * D:(h + 1) * D]
            bv_g[h * (D + 1):h * (D + 1) + D] = bv[rows][h * D:(h + 1) * D]
            bv_g[h * (D + 1) + D] = 1.0
        wo_g = np.ascontiguousarray(Wo[:, rows].T).astype(f32, copy=False)
        per_g.append((wq_g, bq_g, wk_g, bk_g, wv_g, bv_g, wo_g))

    in_maps = []
    for b in range(B):
        for g in range(2):
            wq_g, bq_g, wk_g, bk_g, wv_g, bv_g, wo_g = per_g[g]
            in_maps.append({
                "xT": xT[b], "wq": wq_g, "wk": wk_g, "wv": wv_g, "wo": wo_g,
                "bq": bq_g, "bk": bk_g, "bv": bv_g, "wexp": mask_w[b],
            })
    return in_maps


def kernel(x, mask, Wq, bq, Wk, bk, Wv, bv, Wo, bo, _results_out=None):
    x = np.asarray(x, dtype=np.float32)
    mask = np.asarray(mask, dtype=np.float32)
    B, S, E = x.shape
    nc = _get_nc(S, E, Wq.shape[0] // 2)
    in_maps = make_in_maps(x, mask, np.asarray(Wq), np.asarray(bq),
                           np.asarray(Wk), np.asarray(bk),
                           np.asarray(Wv), np.asarray(bv), np.asarray(Wo))
    res = run_bass_kernel_spmd(nc, in_maps, core_ids=list(range(N_CORES)))
    if _results_out is not None:
        _results_out.append(res)
    out = np.empty((B, S, E), dtype=np.float32)
    bo32 = np.asarray(bo, dtype=np.float32)
    for b in range(B):
        out[b] = res.results[2 * b]["out"] + res.results[2 * b + 1]["out"] + bo32
    return out
